# revision 7
# baseline (speedup 1.0000x reference)
"""Trainium2 Bass kernel for nn_Adapter (ViT video adapter block).

Reference computation (per clip of T=16 frames, 14x14 spatial, 768 ch):
  h   = fc1(x_tokens)                                  # [3136, 384]
  g   = (dw3d_311(h) + dw3d_133(h) + dw3d_333(h))/3 + h
  f   = g + dw3d_311(g)            (proj)
  out = x_tokens + fc2(f)
CLS token passes through unchanged.

Strategy: data-parallel over the 8 clips (B=8), one clip per NeuronCore.
The depthwise 3D convs run entirely on TensorE in a t-on-partition
layout: 48 groups of (16 frames x 8 channels) partitions, spatial plane
on the free dim (zero-halo padded [15 rows x 16 cols]).  The 3 taps
along T collapse into the matmul contraction as block-tridiagonal
128x128 stationaries, so stage 1 is 9 matmuls per group (one per
(dh,dw) window) and stage 2 (proj) is a single banded matmul per group.
Layout transposes between channel-major (fc1/fc2) and t-major (conv)
bounce through DRAM on a single DMA queue (FIFO order enforces the
read-after-write).  fc1/fc2 run on TensorE in bf16; the residual x-add
rides the fc2 PSUM group as an identity matmul.  Output returns bf16.
"""

import os
import sys

sys.path.insert(0, "/opt/trn_rl_repo")

import numpy as np
import ml_dtypes

import concourse.bass as bass
import concourse.mybir as mybir
from concourse import bacc
from concourse.tile import TileContext
from concourse.bass_utils import run_bass_kernel_spmd
from concourse.masks import make_identity


def _install_ntff_hook():
    """Provide antenv.axon_hooks (NTFF profiling hook) if the image lacks
    it, so run_bass_kernel_spmd(trace=True) works.  No-op when present."""
    try:
        import antenv.axon_hooks  # noqa: F401
        return
    except ImportError:
        pass
    import contextlib
    import ctypes
    import types

    so = "/opt/axon/libaxon_pjrt.so"
    try:
        lib = ctypes.CDLL(so)
        lib.axon_start_nrt_profile.argtypes = [
            ctypes.POINTER(ctypes.c_int64), ctypes.c_size_t]
        lib.axon_start_nrt_profile.restype = ctypes.c_int64
        lib.axon_stop_nrt_profile.argtypes = [ctypes.c_char_p]
        lib.axon_stop_nrt_profile.restype = ctypes.c_int64
    except (OSError, AttributeError):
        lib = None

    @contextlib.contextmanager
    def _hook(output_dir, device_ids):
        import jax
        jax.devices()
        ids = ((ctypes.c_int64 * len(device_ids))(*device_ids)
               if device_ids else None)
        rc = lib.axon_start_nrt_profile(ids, len(device_ids or []))
        if rc != 0:
            raise RuntimeError(f"axon_start_nrt_profile rc={rc}")
        try:
            yield
        finally:
            lib.axon_stop_nrt_profile(str(output_dir).encode())

    hook = [_hook if lib is not None else None]
    mod = types.ModuleType("antenv.axon_hooks")
    mod.get_axon_ntff_profile_hook = lambda: hook[0]
    mod.set_axon_ntff_profile_hook = lambda h: hook.__setitem__(0, h)
    sys.modules["antenv.axon_hooks"] = mod


_install_ntff_hook()

BF16 = ml_dtypes.bfloat16

# Problem geometry (hardcoded; kernel must be self-contained).
B, T, L, C, CA = 8, 16, 197, 768, 384
H = W = 14
HW = H * W            # 196
TOK = T * HW          # 3136 tokens per clip
NCORES = 8
NG = 48               # conv groups: 8 channels x 16 frames = 128 partitions
# per-group padded plane: [halo row + 14 data rows] x [14 data + 2 halo]
GH, GW = 15, 16
GPLANE = GH * GW      # 240
PADG = 32             # lead/trail pad inside a group slot (max |off| = 17)
GSLOT = PADG + GPLANE + PADG   # 304
NCH, CHT = 8, 392     # fc1 output chunking

F32 = mybir.dt.float32
BF = mybir.dt.bfloat16

WINS = [(dh, dw) for dh in (-1, 0, 1) for dw in (-1, 0, 1)]

_CACHE = {}

TRACE = os.environ.get("BASS_KERNEL_TRACE", "0") == "1"
LAST_EXEC_NS = None
LAST_RESULTS = None


def _build_graph():
    nc = bacc.Bacc("TRN2", target_bir_lowering=False, debug=False,
                   num_devices=NCORES)

    xT = nc.dram_tensor("xT", [C, TOK], BF, kind="ExternalInput").ap()
    xtok = nc.dram_tensor("xtok", [TOK, C], BF, kind="ExternalInput").ap()
    fc1_wT = nc.dram_tensor("fc1_wT", [C, CA], BF, kind="ExternalInput").ap()
    fc2_wT = nc.dram_tensor("fc2_wT", [CA, C], BF, kind="ExternalInput").ap()
    band1 = nc.dram_tensor("band1", [NG, 128, 9, 128], BF,
                           kind="ExternalInput").ap()
    band2 = nc.dram_tensor("band2", [128, NG * 128], BF,
                           kind="ExternalInput").ap()
    b1g = nc.dram_tensor("b1g", [128, NG], F32, kind="ExternalInput").ap()
    b2g = nc.dram_tensor("b2g", [128, NG], F32, kind="ExternalInput").ap()
    h_dram = nc.dram_tensor("h_dram", [3, 128, T * 224], BF, kind="Internal").ap()
    f_dram = nc.dram_tensor("f_dram", [3, 128, TOK], BF, kind="Internal").ap()
    out = nc.dram_tensor("out", [TOK, C], BF, kind="ExternalOutput").ap()

    mult = mybir.AluOpType.mult
    add = mybir.AluOpType.add
    IDENT = mybir.ActivationFunctionType.Identity

    with TileContext(nc) as tc:
        with (
            tc.tile_pool(name="persist", bufs=1) as pp,
            tc.tile_pool(name="hh", bufs=2) as hhp,
            tc.tile_pool(name="bg", bufs=4) as bgp,
            tc.tile_pool(name="fg", bufs=2) as fgp,
            tc.tile_pool(name="xm", bufs=6) as xmp,
            tc.tile_pool(name="outs", bufs=3) as outp,
            tc.tile_pool(name="ps1", bufs=2, space="PSUM") as ps1p,
            tc.tile_pool(name="ps2", bufs=2, space="PSUM") as ps2p,
            tc.tile_pool(name="psc", bufs=4, space="PSUM") as pscp,
        ):
            # ---- fc1 weights + chunked xT DMA (n-major so fc1 n=0
            # unblocks after the first 6 small transfers) ----
            fc1w_sb = []
            for k in range(6):
                t = pp.tile([128, CA], BF, tag=f"fc1w{k}", name=f"fc1w{k}")
                nc.gpsimd.dma_start(out=t[:],
                                    in_=fc1_wT[k * 128:(k + 1) * 128, :])
                fc1w_sb.append(t)
            xT_sb = [pp.tile([128, TOK], BF, tag=f"xT{k}", name=f"xT{k}")
                     for k in range(6)]
            for n in range(0, NCH, 2):
                for k in range(6):
                    eng = nc.sync if k % 2 == 0 else nc.scalar
                    eng.dma_start(
                        out=xT_sb[k][:, n * CHT:(n + 2) * CHT],
                        in_=xT[k * 128:(k + 1) * 128, n * CHT:(n + 2) * CHT])

            b1_sb = pp.tile([128, NG], F32, tag="b1g", name="b1g_sb")
            nc.sync.dma_start(out=b1_sb[:], in_=b1g)
            b2_sb = pp.tile([128, NG], F32, tag="b2g", name="b2g_sb")
            nc.sync.dma_start(out=b2_sb[:], in_=b2g)
            A_sb = pp.tile([128, NG * 128], BF, tag="band2", name="A_sb")
            nc.gpsimd.dma_start(out=A_sb[:], in_=band2)

            ident = pp.tile([128, 128], BF, tag="ident", name="ident")
            make_identity(nc, ident[:])
            fc2w_sb = []
            for k in range(3):
                t = pp.tile([128, C], BF, tag=f"fc2w{k}", name=f"fc2w{k}")
                nc.gpsimd.dma_start(out=t[:], in_=fc2_wT[k * 128:(k + 1) * 128, :])
                fc2w_sb.append(t)

            # ---- conv buffers (t-major): halo cells must be zero ----
            G = pp.tile([128, NG * GSLOT], BF, tag="G", name="G")
            G2 = pp.tile([128, NG * GSLOT], BF, tag="G2", name="G2")
            nc.scalar.memzero(G[:])
            nc.scalar.memzero(G2[:])
            Ft = [pp.tile([128, TOK], BF, tag=f"Ft_{j}", name=f"Ft_{j}")
                  for j in range(3)]

            def data_cells(buf, base):
                """[14,14] data-cell view of a plane at flat `base`."""
                w = buf[:, base + GW:base + GPLANE]
                return w.rearrange("p (h w) -> p h w", h=H, w=GW)[:, :, 0:W]

            def fc1_block(j):
                # padded 16-wide rows: halo cols carry zeros into G
                hj = hhp.tile([128, T * 224], BF, tag="hjt", name=f"H{j}")
                nc.scalar.memzero(hj[:])
                for n in range(NCH):
                    ps = ps1p.tile([128, CHT], F32, tag="ps1t",
                                   name=f"ps1_{j}_{n}")
                    for k in range(6):
                        nc.tensor.matmul(
                            ps[:],
                            fc1w_sb[k][:, j * 128:(j + 1) * 128],
                            xT_sb[k][:, n * CHT:(n + 1) * CHT],
                            start=(k == 0), stop=(k == 5),
                        )
                    dst = hj[:, n * 448:(n + 1) * 448].rearrange(
                        "p (t h w) -> p t h w", t=2, h=H, w=GW)[:, :, :, 0:W]
                    nc.scalar.copy(
                        dst, ps[:].rearrange("p (t h w) -> p t h w",
                                             t=2, h=H, w=W))
                eng = nc.gpsimd if j < 2 else nc.scalar
                eng.dma_start(out=h_dram[j], in_=hj[:])

            def g_load4(g0):
                """DRAM h -> t-major G data cells for groups g0..g0+3."""
                j, cb = g0 // 16, (g0 % 16) * 8
                src = h_dram[j, cb:cb + 32, :].rearrange(
                    "(sl c) (t r) -> (c t) sl r", sl=4, c=8, t=T, r=224)
                dst = G[:, g0 * GSLOT:(g0 + 4) * GSLOT].rearrange(
                    "p (sl q) -> p sl q", sl=4, q=GSLOT)[
                    :, :, PADG + GW:PADG + GPLANE]
                eng = nc.gpsimd if j < 2 else nc.scalar
                eng.dma_start(out=dst, in_=src)

            def conv_group(g, Bg, r):
                gb = g * GSLOT
                pc = pscp.tile([128, GPLANE], F32, tag="psct", name=f"pc1_{g}")
                for i in range(9):
                    dh, dw = WINS[i]
                    off = dh * GW + dw
                    nc.tensor.matmul(
                        pc[:], Bg[:, r, i * 128:(i + 1) * 128],
                        G[:, gb + PADG + off:gb + PADG + off + GPLANE],
                        start=(i == 0), stop=(i == 8),
                    )
                nc.scalar.activation(
                    data_cells(G2[:], gb + PADG),
                    pc[:, GW:].rearrange("p (h w) -> p h w",
                                         h=H, w=GW)[:, :, 0:W],
                    IDENT, bias=b1_sb[:, g:g + 1], scale=1.0)
                # stage 2 (proj along T): one banded matmul
                pc2 = pscp.tile([128, GPLANE], F32, tag="psct", name=f"pc2_{g}")
                nc.tensor.matmul(
                    pc2[:], A_sb[:, g * 128:(g + 1) * 128],
                    G2[:, gb + PADG:gb + PADG + GPLANE],
                    start=True, stop=True,
                )
                return pc2

            def stage2_evict(g, pc2, f8):
                nc.vector.scalar_tensor_tensor(
                    f8[:, (g % 8) * HW:(g % 8 + 1) * HW].rearrange(
                        "p (h w) -> p h w", h=H, w=W),
                    pc2[:, GW:].rearrange("p (h w) -> p h w",
                                          h=H, w=GW)[:, :, 0:W],
                    1.0,
                    b2_sb[:, g:g + 1].broadcast_to([128, H, W]),
                    op0=mult, op1=add)

            def f_flush(b, f8):
                """Write 8 groups of f to DRAM, read back channel-major."""
                j, cb = b // 2, (b % 2) * 64
                dst = f_dram[j, cb:cb + 64, :].rearrange(
                    "(sl c) (t s) -> (c t) sl s", sl=8, c=8, t=T, s=HW)
                nc.gpsimd.dma_start(
                    out=dst,
                    in_=f8[:].rearrange("p (sl s) -> p sl s", sl=8, s=HW))
                nc.gpsimd.dma_start(
                    out=Ft[j][cb:cb + 64, :], in_=f_dram[j, cb:cb + 64, :])

            def conv_range(g0, g1):
                f8 = None
                for q in range(g0 // 2, g1 // 2):
                    Bg = bgp.tile([128, 2, 9 * 128], BF, tag="bgt", name=f"Bg_{q}")
                    nc.sync.dma_start(
                        out=Bg[:],
                        in_=band1[2 * q:2 * q + 2].rearrange(
                            "g k w m -> k g (w m)"))
                    for r in range(2):
                        g = 2 * q + r
                        if g % 4 == 0:
                            g_load4(g)
                        if g % 8 == 0:
                            f8 = fgp.tile([128, 8 * HW], BF, tag="f8t", name=f"f8_{g}")
                        pc2 = conv_group(g, Bg, r)
                        stage2_evict(g, pc2, f8)
                        if g % 8 == 7:
                            f_flush(g // 8, f8)

            fc1_block(0)
            fc1_block(1)
            conv_range(0, 16)
            fc1_block(2)
            conv_range(16, 32)
            conv_range(32, 48)

            # ---- fc2 + residual add (identity matmul) + store ----
            m_tiles = [(m * 128, 128) for m in range(24)] + [(3072, 64)]
            for (m0, M) in m_tiles:
                xm = xmp.tile([128, C], BF)
                nc.gpsimd.dma_start(out=xm[:M], in_=xtok[m0:m0 + M, :])
                ot = outp.tile([128, C], BF)
                for nh in range(2):
                    ps = ps2p.tile([128, 384], F32, tag="ps2t",
                                   name=f"ps2_{m0}_{nh}")
                    for k in range(3):
                        nc.tensor.matmul(
                            ps[:M],
                            Ft[k][:, m0:m0 + M],
                            fc2w_sb[k][:, nh * 384:(nh + 1) * 384],
                            start=(k == 0), stop=False,
                        )
                    nc.tensor.matmul(
                        ps[:M], ident[:M, :M],
                        xm[:M, nh * 384:(nh + 1) * 384],
                        start=False, stop=True,
                    )
                    if nh == 0:
                        nc.scalar.copy(ot[:M, 0:384], ps[:M])
                    else:
                        nc.vector.tensor_copy(ot[:M, 384:768], ps[:M])
                nc.sync.dma_start(out=out[m0:m0 + M, :], in_=ot[:M])

    nc.compile()
    return nc


def _prep_shared(fc1_w, fc1_b, conv1_w, conv1_b, conv2_w, conv2_b,
                 conv3_w, conv3_b, proj_w, proj_b, fc2_w, fc2_b):
    assert not np.any(fc1_b), "nonzero fc1_b not supported by this build"
    # merged stage-1 kernel: (c1 + c2 + c3)/3 + identity
    w_eff = np.array(conv3_w[:, 0], dtype=np.float64)            # [CA,3,3,3]
    w_eff[:, :, 1, 1] += conv1_w[:, 0, :, 0, 0]
    w_eff[:, 1, :, :] += conv2_w[:, 0, 0, :, :]
    w_eff /= 3.0
    w_eff[:, 1, 1, 1] += 1.0
    b_eff = np.asarray((conv1_b + conv2_b + conv3_b) / 3.0, np.float64)
    # stage-2 (proj) taps along T + identity
    a_eff = np.array(proj_w[:, 0, :, 0, 0], dtype=np.float64)    # [CA,3]
    a_eff[:, 1] += 1.0

    # block-tridiagonal stationaries: band[k=(c8,t_in), m=(c8,t_out)]
    tt = np.arange(T)
    dtm = tt[:, None] - tt[None, :]                  # t_in - t_out
    mask = np.abs(dtm) <= 1
    dtc = np.clip(dtm + 1, 0, 2)
    band1_arr = np.zeros((NG, 128, 9, 128), np.float32)
    band2_arr = np.zeros((128, NG, 128), np.float32)
    for g in range(NG):
        for c8 in range(8):
            c = 8 * g + c8
            s = slice(c8 * T, (c8 + 1) * T)
            for i, (dh, dw) in enumerate(WINS):
                blk = w_eff[c, dtc, dh + 1, dw + 1] * mask
                band1_arr[g, s, i, s] = blk
            band2_arr[s, g, s] = a_eff[c, dtc] * mask
    band2_arr = band2_arr.reshape(128, NG * 128)

    # per-partition biases in group layout: partition (c8, t) -> channel 8g+c8
    cidx = (np.arange(128) // T)                     # c8 of each partition
    b1g_arr = np.empty((128, NG), np.float32)
    b2g_arr = np.empty((128, NG), np.float32)
    for g in range(NG):
        b1g_arr[:, g] = b_eff[8 * g + cidx]
        b2g_arr[:, g] = np.asarray(proj_b, np.float64)[8 * g + cidx]

    fc1_wT = np.ascontiguousarray(np.asarray(fc1_w, np.float32).T).astype(BF16)
    fc2_wT = np.ascontiguousarray(np.asarray(fc2_w, np.float32).T).astype(BF16)
    return dict(fc1_wT=fc1_wT, fc2_wT=fc2_wT,
                band1=band1_arr.astype(BF16), band2=band2_arr.astype(BF16),
                b1g=b1g_arr, b2g=b2g_arr), np.asarray(fc2_b, np.float32)


def kernel(x, fc1_w, fc1_b, conv1_w, conv1_b, conv2_w, conv2_b,
           conv3_w, conv3_b, proj_w, proj_b, fc2_w, fc2_b, T=16):
    global LAST_EXEC_NS, LAST_RESULTS
    x = np.asarray(x, np.float32)
    Tv = int(np.asarray(T))
    assert Tv == 16 and x.shape == (B * Tv, L, C), (Tv, x.shape)

    if "nc" not in _CACHE:
        _CACHE["nc"] = _build_graph()
    nc = _CACHE["nc"]

    shared, fc2_b_np = _prep_shared(
        np.asarray(fc1_w, np.float32), np.asarray(fc1_b, np.float32),
        np.asarray(conv1_w, np.float32), np.asarray(conv1_b, np.float32),
        np.asarray(conv2_w, np.float32), np.asarray(conv2_b, np.float32),
        np.asarray(conv3_w, np.float32), np.asarray(conv3_b, np.float32),
        np.asarray(proj_w, np.float32), np.asarray(proj_b, np.float32),
        np.asarray(fc2_w, np.float32), np.asarray(fc2_b, np.float32))

    in_maps = []
    for i in range(NCORES):
        clip = x[i * Tv:(i + 1) * Tv]                    # [16, 197, 768]
        tokens = np.ascontiguousarray(clip[:, 1:, :]).reshape(TOK, C)
        m = dict(shared)
        m["xT"] = np.ascontiguousarray(tokens.T).astype(BF16)
        m["xtok"] = (tokens + fc2_b_np[None, :]).astype(BF16)
        in_maps.append(m)

    res = run_bass_kernel_spmd(nc, in_maps, core_ids=list(range(NCORES)),
                               trace=TRACE)
    LAST_EXEC_NS = res.exec_time_ns
    LAST_RESULTS = res

    full = np.array(x)  # CLS rows (and everything) start as x
    for i in range(NCORES):
        h = res.results[i]["out"].astype(np.float32).reshape(Tv, HW, C)
        full[i * Tv:(i + 1) * Tv, 1:, :] = h
    return full


# revision 10
# speedup vs baseline: 1.3116x; 1.3116x over previous
"""Trainium2 Bass kernel for nn_Adapter (ViT video adapter block).

Reference computation (per clip of T=16 frames, 14x14 spatial, 768 ch):
  h   = fc1(x_tokens)                                  # [3136, 384]
  g   = (dw3d_311(h) + dw3d_133(h) + dw3d_333(h))/3 + h
  f   = g + dw3d_311(g)            (proj)
  out = x_tokens + fc2(f)
CLS token passes through unchanged.

Strategy: data-parallel over the 8 clips (B=8), one clip per NeuronCore.
The depthwise 3D convs run entirely on TensorE in a t-on-partition
layout: 48 groups of (16 frames x 8 channels) partitions, spatial plane
on the free dim (zero-halo padded [15 rows x 16 cols]).  The 3 taps
along T collapse into the matmul contraction as block-tridiagonal
128x128 stationaries, so stage 1 is 9 matmuls per group (one per
(dh,dw) window) and stage 2 (proj) is a single banded matmul per group.
Layout transposes between channel-major (fc1/fc2) and t-major (conv)
bounce through DRAM on a single DMA queue (FIFO order enforces the
read-after-write).  fc1/fc2 run on TensorE in bf16; the residual x-add
rides the fc2 PSUM group as an identity matmul.  Output returns bf16.
"""

import os
import sys

sys.path.insert(0, "/opt/trn_rl_repo")

import numpy as np
import ml_dtypes

import concourse.bass as bass
import concourse.mybir as mybir
from concourse import bacc
from concourse.tile import TileContext
from concourse.bass_utils import run_bass_kernel_spmd
from concourse.masks import make_identity


def _install_ntff_hook():
    """Provide antenv.axon_hooks (NTFF profiling hook) if the image lacks
    it, so run_bass_kernel_spmd(trace=True) works.  No-op when present."""
    try:
        import antenv.axon_hooks  # noqa: F401
        return
    except ImportError:
        pass
    import contextlib
    import ctypes
    import types

    so = "/opt/axon/libaxon_pjrt.so"
    try:
        lib = ctypes.CDLL(so)
        lib.axon_start_nrt_profile.argtypes = [
            ctypes.POINTER(ctypes.c_int64), ctypes.c_size_t]
        lib.axon_start_nrt_profile.restype = ctypes.c_int64
        lib.axon_stop_nrt_profile.argtypes = [ctypes.c_char_p]
        lib.axon_stop_nrt_profile.restype = ctypes.c_int64
    except (OSError, AttributeError):
        lib = None

    @contextlib.contextmanager
    def _hook(output_dir, device_ids):
        import jax
        jax.devices()
        ids = ((ctypes.c_int64 * len(device_ids))(*device_ids)
               if device_ids else None)
        rc = lib.axon_start_nrt_profile(ids, len(device_ids or []))
        if rc != 0:
            raise RuntimeError(f"axon_start_nrt_profile rc={rc}")
        try:
            yield
        finally:
            lib.axon_stop_nrt_profile(str(output_dir).encode())

    hook = [_hook if lib is not None else None]
    mod = types.ModuleType("antenv.axon_hooks")
    mod.get_axon_ntff_profile_hook = lambda: hook[0]
    mod.set_axon_ntff_profile_hook = lambda h: hook.__setitem__(0, h)
    sys.modules["antenv.axon_hooks"] = mod


_install_ntff_hook()

BF16 = ml_dtypes.bfloat16

# Problem geometry (hardcoded; kernel must be self-contained).
B, T, L, C, CA = 8, 16, 197, 768, 384
H = W = 14
HW = H * W            # 196
TOK = T * HW          # 3136 tokens per clip
NCORES = 8
NG = 48               # conv groups: 8 channels x 16 frames = 128 partitions
# per-group padded plane: [halo row + 14 data rows] x [14 data + 2 halo]
GH, GW = 15, 16
GPLANE = GH * GW      # 240
PADG = 32             # lead/trail pad inside a group slot (max |off| = 17)
GSLOT = PADG + GPLANE + PADG   # 304
NCH, CHT = 8, 392     # fc1 output chunking

F32 = mybir.dt.float32
BF = mybir.dt.bfloat16

WINS = [(dh, dw) for dh in (-1, 0, 1) for dw in (-1, 0, 1)]

_CACHE = {}

TRACE = os.environ.get("BASS_KERNEL_TRACE", "0") == "1"
LAST_EXEC_NS = None
LAST_RESULTS = None


def _build_graph():
    nc = bacc.Bacc("TRN2", target_bir_lowering=False, debug=False,
                   num_devices=NCORES)

    xT = nc.dram_tensor("xT", [C, TOK], BF, kind="ExternalInput").ap()
    xtok = nc.dram_tensor("xtok", [TOK, C], BF, kind="ExternalInput").ap()
    fc1_wT = nc.dram_tensor("fc1_wT", [C, CA], BF, kind="ExternalInput").ap()
    fc2_wT = nc.dram_tensor("fc2_wT", [CA, C], BF, kind="ExternalInput").ap()
    band1 = nc.dram_tensor("band1", [NG, 128, 9, 128], BF,
                           kind="ExternalInput").ap()
    bcg = nc.dram_tensor("bcg", [128, NG], F32, kind="ExternalInput").ap()
    h_dram = nc.dram_tensor("h_dram", [3, 128, T * 224], BF, kind="Internal").ap()
    f_dram = nc.dram_tensor("f_dram", [3, 128, TOK], BF, kind="Internal").ap()
    out = nc.dram_tensor("out", [TOK, C], BF, kind="ExternalOutput").ap()

    mult = mybir.AluOpType.mult
    add = mybir.AluOpType.add
    IDENT = mybir.ActivationFunctionType.Identity

    with TileContext(nc) as tc:
        with (
            tc.tile_pool(name="persist", bufs=1) as pp,
            tc.tile_pool(name="hh", bufs=2) as hhp,
            tc.tile_pool(name="bg", bufs=4) as bgp,
            tc.tile_pool(name="fg", bufs=2) as fgp,
            tc.tile_pool(name="xm", bufs=6) as xmp,
            tc.tile_pool(name="outs", bufs=3) as outp,
            tc.tile_pool(name="ps1", bufs=2, space="PSUM") as ps1p,
            tc.tile_pool(name="ps2", bufs=2, space="PSUM") as ps2p,
            tc.tile_pool(name="psc", bufs=4, space="PSUM") as pscp,
        ):
            # ---- fc1 weights + chunked xT DMA (n-major so fc1 n=0
            # unblocks after the first 6 small transfers) ----
            fc1w_sb = []
            for k in range(6):
                t = pp.tile([128, CA], BF, tag=f"fc1w{k}", name=f"fc1w{k}")
                nc.gpsimd.dma_start(out=t[:],
                                    in_=fc1_wT[k * 128:(k + 1) * 128, :])
                fc1w_sb.append(t)
            xT_sb = [pp.tile([128, TOK], BF, tag=f"xT{k}", name=f"xT{k}")
                     for k in range(6)]
            for n in range(0, NCH, 4):
                for k in range(6):
                    nc.sync.dma_start(
                        out=xT_sb[k][:, n * CHT:(n + 4) * CHT],
                        in_=xT[k * 128:(k + 1) * 128, n * CHT:(n + 4) * CHT])

            bc_sb = pp.tile([128, NG], F32, tag="bcg", name="bc_sb")
            nc.sync.dma_start(out=bc_sb[:], in_=bcg)

            ident = pp.tile([128, 128], BF, tag="ident", name="ident")
            make_identity(nc, ident[:])
            fc2w_sb = []
            for k in range(3):
                t = pp.tile([128, C], BF, tag=f"fc2w{k}", name=f"fc2w{k}")
                nc.gpsimd.dma_start(out=t[:], in_=fc2_wT[k * 128:(k + 1) * 128, :])
                fc2w_sb.append(t)

            # ---- conv buffers (t-major): halo cells must be zero ----
            G = pp.tile([128, NG * GSLOT], BF, tag="G", name="G")
            nc.vector.memzero(G[:])
            Ft = [pp.tile([128, TOK], BF, tag=f"Ft_{j}", name=f"Ft_{j}")
                  for j in range(3)]

            def data_cells(buf, base):
                """[14,14] data-cell view of a plane at flat `base`."""
                w = buf[:, base + GW:base + GPLANE]
                return w.rearrange("p (h w) -> p h w", h=H, w=GW)[:, :, 0:W]

            def fc1_block(j):
                # padded 16-wide rows: halo cols carry zeros into G
                hj = hhp.tile([128, T * 224], BF, tag="hjt", name=f"H{j}")
                if j < 2:
                    nc.vector.memzero(hj[:])
                for n in range(NCH):
                    ps = ps1p.tile([128, CHT], F32, tag="ps1t",
                                   name=f"ps1_{j}_{n}")
                    for k in range(6):
                        nc.tensor.matmul(
                            ps[:],
                            fc1w_sb[k][:, j * 128:(j + 1) * 128],
                            xT_sb[k][:, n * CHT:(n + 1) * CHT],
                            start=(k == 0), stop=(k == 5),
                        )
                    dst = hj[:, n * 448:(n + 1) * 448].rearrange(
                        "p (t h w) -> p t h w", t=2, h=H, w=GW)[:, :, :, 0:W]
                    nc.scalar.copy(
                        dst, ps[:].rearrange("p (t h w) -> p t h w",
                                             t=2, h=H, w=W))
                return hj

            def g_load4(g0):
                """DRAM h -> t-major G data cells for groups g0..g0+3."""
                j, cb = g0 // 16, (g0 % 16) * 8
                src = h_dram[j, cb:cb + 32, :].rearrange(
                    "(sl c) (t r) -> (c t) sl r", sl=4, c=8, t=T, r=224)
                dst = G[:, g0 * GSLOT:(g0 + 4) * GSLOT].rearrange(
                    "p (sl q) -> p sl q", sl=4, q=GSLOT)[
                    :, :, PADG + GW:PADG + GPLANE]
                nc.gpsimd.dma_start(out=dst, in_=src)

            def conv_group(g, Bg, r):
                gb = g * GSLOT
                pc = pscp.tile([128, GPLANE], F32, tag="psct", name=f"pc1_{g}")
                for i in range(9):
                    dh, dw = WINS[i]
                    off = dh * GW + dw
                    nc.tensor.matmul(
                        pc[:], Bg[:, r, i * 128:(i + 1) * 128],
                        G[:, gb + PADG + off:gb + PADG + off + GPLANE],
                        start=(i == 0), stop=(i == 8),
                    )
                return pc

            def stage2_evict(g, pc2, f8):
                nc.vector.scalar_tensor_tensor(
                    f8[:, (g % 8) * HW:(g % 8 + 1) * HW].rearrange(
                        "p (h w) -> p h w", h=H, w=W),
                    pc2[:, GW:].rearrange("p (h w) -> p h w",
                                          h=H, w=GW)[:, :, 0:W],
                    1.0,
                    bc_sb[:, g:g + 1].broadcast_to([128, H, W]),
                    op0=mult, op1=add)

            def f_flush(b, f8):
                """Write 8 groups of f to DRAM, read back channel-major."""
                j, cb = b // 2, (b % 2) * 64
                dst = f_dram[j, cb:cb + 64, :].rearrange(
                    "(sl c) (t s) -> (c t) sl s", sl=8, c=8, t=T, s=HW)
                nc.scalar.dma_start(
                    out=dst,
                    in_=f8[:].rearrange("p (sl s) -> p sl s", sl=8, s=HW))
                nc.scalar.dma_start(
                    out=Ft[j][cb:cb + 64, :], in_=f_dram[j, cb:cb + 64, :])

            def conv_range(g0, g1):
                f8 = None
                for q in range(g0 // 2, g1 // 2):
                    Bg = bgp.tile([128, 2, 9 * 128], BF, tag="bgt", name=f"Bg_{q}")
                    nc.sync.dma_start(
                        out=Bg[:],
                        in_=band1[2 * q:2 * q + 2].rearrange(
                            "g k w m -> k g (w m)"))
                    for r in range(2):
                        g = 2 * q + r
                        if g % 4 == 0:
                            g_load4(g)
                        if g % 8 == 0:
                            f8 = fgp.tile([128, 8 * HW], BF, tag="f8t", name=f"f8_{g}")
                        pc = conv_group(g, Bg, r)
                        stage2_evict(g, pc, f8)
                        if g % 8 == 7:
                            f_flush(g // 8, f8)

            h0 = fc1_block(0)
            nc.gpsimd.dma_start(out=h_dram[0], in_=h0[:])
            h1 = fc1_block(1)
            conv_range(0, 16)
            nc.gpsimd.dma_start(out=h_dram[1], in_=h1[:])
            h2 = fc1_block(2)
            conv_range(16, 32)
            nc.gpsimd.dma_start(out=h_dram[2], in_=h2[:])
            conv_range(32, 48)

            # ---- fc2 + residual add (identity matmul) + store ----
            m_tiles = [(m * 128, 128) for m in range(24)] + [(3072, 64)]
            for (m0, M) in m_tiles:
                xm = xmp.tile([128, C], BF)
                nc.gpsimd.dma_start(out=xm[:M], in_=xtok[m0:m0 + M, :])
                ot = outp.tile([128, C], BF)
                for nh in range(2):
                    ps = ps2p.tile([128, 384], F32, tag="ps2t",
                                   name=f"ps2_{m0}_{nh}")
                    for k in range(3):
                        nc.tensor.matmul(
                            ps[:M],
                            Ft[k][:, m0:m0 + M],
                            fc2w_sb[k][:, nh * 384:(nh + 1) * 384],
                            start=(k == 0), stop=False,
                        )
                    nc.tensor.matmul(
                        ps[:M], ident[:M, :M],
                        xm[:M, nh * 384:(nh + 1) * 384],
                        start=False, stop=True,
                    )
                    if nh == 0:
                        nc.scalar.copy(ot[:M, 0:384], ps[:M])
                    else:
                        nc.vector.tensor_copy(ot[:M, 384:768], ps[:M])
                nc.sync.dma_start(out=out[m0:m0 + M, :], in_=ot[:M])

    nc.compile()
    return nc


def _prep_shared(fc1_w, fc1_b, conv1_w, conv1_b, conv2_w, conv2_b,
                 conv3_w, conv3_b, proj_w, proj_b, fc2_w, fc2_b):
    assert not np.any(fc1_b), "nonzero fc1_b not supported by this build"
    # merged stage-1 kernel: (c1 + c2 + c3)/3 + identity
    w_eff = np.array(conv3_w[:, 0], dtype=np.float64)            # [CA,3,3,3]
    w_eff[:, :, 1, 1] += conv1_w[:, 0, :, 0, 0]
    w_eff[:, 1, :, :] += conv2_w[:, 0, 0, :, :]
    w_eff /= 3.0
    w_eff[:, 1, 1, 1] += 1.0
    b_eff = np.asarray((conv1_b + conv2_b + conv3_b) / 3.0, np.float64)
    # stage-2 (proj) taps along T + identity
    a_eff = np.array(proj_w[:, 0, :, 0, 0], dtype=np.float64)    # [CA,3]
    a_eff[:, 1] += 1.0

    # block-tridiagonal stationaries: band[k=(c8,t_in), m=(c8,t_out)]
    tt = np.arange(T)
    dtm = tt[:, None] - tt[None, :]                  # t_in - t_out
    mask = np.abs(dtm) <= 1
    dtc = np.clip(dtm + 1, 0, 2)
    # composed per-group stationaries C_w = A_g @ B_{g,w} (stage-2 proj
    # folded into stage 1) and the matching per-partition bias
    band1_arr = np.zeros((NG, 128, 9, 128), np.float64)
    bcg_arr = np.empty((128, NG), np.float64)
    cidx = (np.arange(128) // T)                     # c8 of each partition
    pb = np.asarray(proj_b, np.float64)
    for g in range(NG):
        Ag = np.zeros((128, 128), np.float64)
        for c8 in range(8):
            c = 8 * g + c8
            s = slice(c8 * T, (c8 + 1) * T)
            Ag[s, s] = a_eff[c, dtc] * mask
            for i, (dh, dw) in enumerate(WINS):
                band1_arr[g, s, i, s] = w_eff[c, dtc, dh + 1, dw + 1] * mask
        for i in range(9):
            band1_arr[g, :, i, :] = Ag @ band1_arr[g, :, i, :]
        b1vec = b_eff[8 * g + cidx]
        bcg_arr[:, g] = Ag.T @ b1vec + pb[8 * g + cidx]

    fc1_wT = np.ascontiguousarray(np.asarray(fc1_w, np.float32).T).astype(BF16)
    fc2_wT = np.ascontiguousarray(np.asarray(fc2_w, np.float32).T).astype(BF16)
    return dict(fc1_wT=fc1_wT, fc2_wT=fc2_wT,
                band1=band1_arr.astype(BF16),
                bcg=bcg_arr.astype(np.float32)), np.asarray(fc2_b, np.float32)


def kernel(x, fc1_w, fc1_b, conv1_w, conv1_b, conv2_w, conv2_b,
           conv3_w, conv3_b, proj_w, proj_b, fc2_w, fc2_b, T=16):
    global LAST_EXEC_NS, LAST_RESULTS
    x = np.asarray(x, np.float32)
    Tv = int(np.asarray(T))
    assert Tv == 16 and x.shape == (B * Tv, L, C), (Tv, x.shape)

    if "nc" not in _CACHE:
        _CACHE["nc"] = _build_graph()
    nc = _CACHE["nc"]

    shared, fc2_b_np = _prep_shared(
        np.asarray(fc1_w, np.float32), np.asarray(fc1_b, np.float32),
        np.asarray(conv1_w, np.float32), np.asarray(conv1_b, np.float32),
        np.asarray(conv2_w, np.float32), np.asarray(conv2_b, np.float32),
        np.asarray(conv3_w, np.float32), np.asarray(conv3_b, np.float32),
        np.asarray(proj_w, np.float32), np.asarray(proj_b, np.float32),
        np.asarray(fc2_w, np.float32), np.asarray(fc2_b, np.float32))

    in_maps = []
    for i in range(NCORES):
        clip = x[i * Tv:(i + 1) * Tv]                    # [16, 197, 768]
        tokens = np.ascontiguousarray(clip[:, 1:, :]).reshape(TOK, C)
        m = dict(shared)
        m["xT"] = np.ascontiguousarray(tokens.T).astype(BF16)
        m["xtok"] = (tokens + fc2_b_np[None, :]).astype(BF16)
        in_maps.append(m)

    res = run_bass_kernel_spmd(nc, in_maps, core_ids=list(range(NCORES)),
                               trace=TRACE)
    LAST_EXEC_NS = res.exec_time_ns
    LAST_RESULTS = res

    full = np.array(x)  # CLS rows (and everything) start as x
    for i in range(NCORES):
        h = res.results[i]["out"].astype(np.float32).reshape(Tv, HW, C)
        full[i * Tv:(i + 1) * Tv, 1:, :] = h
    return full


# revision 11
# speedup vs baseline: 1.3217x; 1.0077x over previous
"""Trainium2 Bass kernel for nn_Adapter (ViT video adapter block).

Reference computation (per clip of T=16 frames, 14x14 spatial, 768 ch):
  h   = fc1(x_tokens)                                  # [3136, 384]
  g   = (dw3d_311(h) + dw3d_133(h) + dw3d_333(h))/3 + h
  f   = g + dw3d_311(g)            (proj)
  out = x_tokens + fc2(f)
CLS token passes through unchanged.

Strategy: data-parallel over the 8 clips (B=8), one clip per NeuronCore.
The depthwise 3D convs run entirely on TensorE in a t-on-partition
layout: 48 groups of (16 frames x 8 channels) partitions, spatial plane
on the free dim (zero-halo padded [15 rows x 16 cols]).  The 3 taps
along T collapse into the matmul contraction as block-tridiagonal
128x128 stationaries, so stage 1 is 9 matmuls per group (one per
(dh,dw) window) and stage 2 (proj) is a single banded matmul per group.
Layout transposes between channel-major (fc1/fc2) and t-major (conv)
bounce through DRAM on a single DMA queue (FIFO order enforces the
read-after-write).  fc1/fc2 run on TensorE in bf16; the residual x-add
rides the fc2 PSUM group as an identity matmul.  Output returns bf16.
"""

import os
import sys

sys.path.insert(0, "/opt/trn_rl_repo")

import numpy as np
import ml_dtypes

import concourse.bass as bass
import concourse.mybir as mybir
from concourse import bacc
from concourse.tile import TileContext
from concourse.bass_utils import run_bass_kernel_spmd
from concourse.masks import make_identity


def _install_ntff_hook():
    """Provide antenv.axon_hooks (NTFF profiling hook) if the image lacks
    it, so run_bass_kernel_spmd(trace=True) works.  No-op when present."""
    try:
        import antenv.axon_hooks  # noqa: F401
        return
    except ImportError:
        pass
    import contextlib
    import ctypes
    import types

    so = "/opt/axon/libaxon_pjrt.so"
    try:
        lib = ctypes.CDLL(so)
        lib.axon_start_nrt_profile.argtypes = [
            ctypes.POINTER(ctypes.c_int64), ctypes.c_size_t]
        lib.axon_start_nrt_profile.restype = ctypes.c_int64
        lib.axon_stop_nrt_profile.argtypes = [ctypes.c_char_p]
        lib.axon_stop_nrt_profile.restype = ctypes.c_int64
    except (OSError, AttributeError):
        lib = None

    @contextlib.contextmanager
    def _hook(output_dir, device_ids):
        import jax
        jax.devices()
        ids = ((ctypes.c_int64 * len(device_ids))(*device_ids)
               if device_ids else None)
        rc = lib.axon_start_nrt_profile(ids, len(device_ids or []))
        if rc != 0:
            raise RuntimeError(f"axon_start_nrt_profile rc={rc}")
        try:
            yield
        finally:
            lib.axon_stop_nrt_profile(str(output_dir).encode())

    hook = [_hook if lib is not None else None]
    mod = types.ModuleType("antenv.axon_hooks")
    mod.get_axon_ntff_profile_hook = lambda: hook[0]
    mod.set_axon_ntff_profile_hook = lambda h: hook.__setitem__(0, h)
    sys.modules["antenv.axon_hooks"] = mod


_install_ntff_hook()

BF16 = ml_dtypes.bfloat16

# Problem geometry (hardcoded; kernel must be self-contained).
B, T, L, C, CA = 8, 16, 197, 768, 384
H = W = 14
HW = H * W            # 196
TOK = T * HW          # 3136 tokens per clip
NCORES = 8
NG = 48               # conv groups: 8 channels x 16 frames = 128 partitions
# per-group padded plane: [halo row + 14 data rows] x [14 data + 2 halo]
GH, GW = 15, 16
GPLANE = GH * GW      # 240
PADG = 32             # lead/trail pad inside a group slot (max |off| = 17)
GSLOT = PADG + GPLANE + PADG   # 304
NCH, CHT = 8, 392     # fc1 output chunking

F32 = mybir.dt.float32
BF = mybir.dt.bfloat16

WINS = [(dh, dw) for dh in (-1, 0, 1) for dw in (-1, 0, 1)]

_CACHE = {}

TRACE = os.environ.get("BASS_KERNEL_TRACE", "0") == "1"
LAST_EXEC_NS = None
LAST_RESULTS = None


def _build_graph():
    nc = bacc.Bacc("TRN2", target_bir_lowering=False, debug=False,
                   num_devices=NCORES)

    xT = nc.dram_tensor("xT", [C, TOK], BF, kind="ExternalInput").ap()
    xtok = nc.dram_tensor("xtok", [TOK, C], BF, kind="ExternalInput").ap()
    fc1_wT = nc.dram_tensor("fc1_wT", [C, CA], BF, kind="ExternalInput").ap()
    fc2_wT = nc.dram_tensor("fc2_wT", [CA, C], BF, kind="ExternalInput").ap()
    band1 = nc.dram_tensor("band1", [NG, 128, 9, 128], BF,
                           kind="ExternalInput").ap()
    bcg = nc.dram_tensor("bcg", [128, NG], F32, kind="ExternalInput").ap()
    h_dram = nc.dram_tensor("h_dram", [3, 128, T * 224], BF, kind="Internal").ap()
    f_dram = nc.dram_tensor("f_dram", [3, 128, TOK], BF, kind="Internal").ap()
    out = nc.dram_tensor("out", [TOK, C], BF, kind="ExternalOutput").ap()

    mult = mybir.AluOpType.mult
    add = mybir.AluOpType.add
    IDENT = mybir.ActivationFunctionType.Identity

    with TileContext(nc) as tc:
        with (
            tc.tile_pool(name="persist", bufs=1) as pp,
            tc.tile_pool(name="hh", bufs=2) as hhp,
            tc.tile_pool(name="bg", bufs=6) as bgp,
            tc.tile_pool(name="fg", bufs=3) as fgp,
            tc.tile_pool(name="xm", bufs=6) as xmp,
            tc.tile_pool(name="outs", bufs=3) as outp,
            tc.tile_pool(name="ps1", bufs=2, space="PSUM") as ps1p,
            tc.tile_pool(name="ps2", bufs=2, space="PSUM") as ps2p,
            tc.tile_pool(name="psc", bufs=4, space="PSUM") as pscp,
        ):
            # ---- fc1 weights + chunked xT DMA (n-major so fc1 n=0
            # unblocks after the first 6 small transfers) ----
            fc1w_sb = []
            for k in range(6):
                t = pp.tile([128, CA], BF, tag=f"fc1w{k}", name=f"fc1w{k}")
                nc.gpsimd.dma_start(out=t[:],
                                    in_=fc1_wT[k * 128:(k + 1) * 128, :])
                fc1w_sb.append(t)
            xT_sb = [pp.tile([128, TOK], BF, tag=f"xT{k}", name=f"xT{k}")
                     for k in range(6)]
            for n0, n1 in ((0, 2), (2, 4), (4, 8)):
                for k in range(6):
                    nc.sync.dma_start(
                        out=xT_sb[k][:, n0 * CHT:n1 * CHT],
                        in_=xT[k * 128:(k + 1) * 128, n0 * CHT:n1 * CHT])

            bc_sb = pp.tile([128, NG], F32, tag="bcg", name="bc_sb")
            nc.sync.dma_start(out=bc_sb[:], in_=bcg)

            ident = pp.tile([128, 128], BF, tag="ident", name="ident")
            make_identity(nc, ident[:])
            fc2w_sb = []
            for k in range(3):
                t = pp.tile([128, C], BF, tag=f"fc2w{k}", name=f"fc2w{k}")
                nc.gpsimd.dma_start(out=t[:], in_=fc2_wT[k * 128:(k + 1) * 128, :])
                fc2w_sb.append(t)

            # ---- conv buffers (t-major): halo cells must be zero ----
            G = pp.tile([128, NG * GSLOT], BF, tag="G", name="G")
            nc.vector.memzero(G[:])
            Ft = [pp.tile([128, TOK], BF, tag=f"Ft_{j}", name=f"Ft_{j}")
                  for j in range(3)]

            def data_cells(buf, base):
                """[14,14] data-cell view of a plane at flat `base`."""
                w = buf[:, base + GW:base + GPLANE]
                return w.rearrange("p (h w) -> p h w", h=H, w=GW)[:, :, 0:W]

            def fc1_block(j):
                # padded 16-wide rows: halo cols carry zeros into G
                hj = hhp.tile([128, T * 224], BF, tag="hjt", name=f"H{j}")
                if j < 2:
                    nc.vector.memzero(hj[:])
                for n in range(NCH):
                    ps = ps1p.tile([128, CHT], F32, tag="ps1t",
                                   name=f"ps1_{j}_{n}")
                    for k in range(6):
                        nc.tensor.matmul(
                            ps[:],
                            fc1w_sb[k][:, j * 128:(j + 1) * 128],
                            xT_sb[k][:, n * CHT:(n + 1) * CHT],
                            start=(k == 0), stop=(k == 5),
                        )
                    dst = hj[:, n * 448:(n + 1) * 448].rearrange(
                        "p (t h w) -> p t h w", t=2, h=H, w=GW)[:, :, :, 0:W]
                    nc.scalar.copy(
                        dst, ps[:].rearrange("p (t h w) -> p t h w",
                                             t=2, h=H, w=W))
                return hj

            def g_load4(g0):
                """DRAM h -> t-major G data cells for groups g0..g0+3."""
                j, cb = g0 // 16, (g0 % 16) * 8
                src = h_dram[j, cb:cb + 32, :].rearrange(
                    "(sl c) (t r) -> (c t) sl r", sl=4, c=8, t=T, r=224)
                dst = G[:, g0 * GSLOT:(g0 + 4) * GSLOT].rearrange(
                    "p (sl q) -> p sl q", sl=4, q=GSLOT)[
                    :, :, PADG + GW:PADG + GPLANE]
                nc.gpsimd.dma_start(out=dst, in_=src)

            def conv_group(g, Bg, r):
                gb = g * GSLOT
                pc = pscp.tile([128, GPLANE], F32, tag="psct", name=f"pc1_{g}")
                for i in range(9):
                    dh, dw = WINS[i]
                    off = dh * GW + dw
                    nc.tensor.matmul(
                        pc[:], Bg[:, r, i * 128:(i + 1) * 128],
                        G[:, gb + PADG + off:gb + PADG + off + GPLANE],
                        start=(i == 0), stop=(i == 8),
                    )
                return pc

            def stage2_evict(g, pc2, f8):
                nc.vector.scalar_tensor_tensor(
                    f8[:, (g % 8) * HW:(g % 8 + 1) * HW].rearrange(
                        "p (h w) -> p h w", h=H, w=W),
                    pc2[:, GW:].rearrange("p (h w) -> p h w",
                                          h=H, w=GW)[:, :, 0:W],
                    1.0,
                    bc_sb[:, g:g + 1].broadcast_to([128, H, W]),
                    op0=mult, op1=add)

            def f_flush(b, f8):
                """Write 8 groups of f to DRAM, read back channel-major."""
                j, cb = b // 2, (b % 2) * 64
                dst = f_dram[j, cb:cb + 64, :].rearrange(
                    "(sl c) (t s) -> (c t) sl s", sl=8, c=8, t=T, s=HW)
                nc.scalar.dma_start(
                    out=dst,
                    in_=f8[:].rearrange("p (sl s) -> p sl s", sl=8, s=HW))
                nc.scalar.dma_start(
                    out=Ft[j][cb:cb + 64, :], in_=f_dram[j, cb:cb + 64, :])

            def conv_range(g0, g1):
                f8 = None
                for q in range(g0 // 2, g1 // 2):
                    Bg = bgp.tile([128, 2, 9 * 128], BF, tag="bgt", name=f"Bg_{q}")
                    nc.sync.dma_start(
                        out=Bg[:],
                        in_=band1[2 * q:2 * q + 2].rearrange(
                            "g k w m -> k g (w m)"))
                    for r in range(2):
                        g = 2 * q + r
                        if g % 4 == 0:
                            g_load4(g)
                        if g % 8 == 0:
                            f8 = fgp.tile([128, 8 * HW], BF, tag="f8t", name=f"f8_{g}")
                        pc = conv_group(g, Bg, r)
                        stage2_evict(g, pc, f8)
                        if g % 8 == 7:
                            f_flush(g // 8, f8)

            h0 = fc1_block(0)
            nc.gpsimd.dma_start(out=h_dram[0], in_=h0[:])
            h1 = fc1_block(1)
            conv_range(0, 16)
            nc.gpsimd.dma_start(out=h_dram[1], in_=h1[:])
            h2 = fc1_block(2)
            conv_range(16, 32)
            nc.gpsimd.dma_start(out=h_dram[2], in_=h2[:])
            conv_range(32, 48)

            # ---- fc2 + residual add (identity matmul) + store ----
            m_tiles = [(m * 128, 128) for m in range(24)] + [(3072, 64)]
            for (m0, M) in m_tiles:
                xm = xmp.tile([128, C], BF)
                nc.gpsimd.dma_start(out=xm[:M], in_=xtok[m0:m0 + M, :])
                ot = outp.tile([128, C], BF)
                for nh in range(2):
                    ps = ps2p.tile([128, 384], F32, tag="ps2t",
                                   name=f"ps2_{m0}_{nh}")
                    for k in range(3):
                        nc.tensor.matmul(
                            ps[:M],
                            Ft[k][:, m0:m0 + M],
                            fc2w_sb[k][:, nh * 384:(nh + 1) * 384],
                            start=(k == 0), stop=False,
                        )
                    nc.tensor.matmul(
                        ps[:M], ident[:M, :M],
                        xm[:M, nh * 384:(nh + 1) * 384],
                        start=False, stop=True,
                    )
                    if nh == 0:
                        nc.scalar.copy(ot[:M, 0:384], ps[:M])
                    else:
                        nc.vector.tensor_copy(ot[:M, 384:768], ps[:M])
                nc.scalar.dma_start(out=out[m0:m0 + M, :], in_=ot[:M])

    nc.compile()
    return nc


def _prep_shared(fc1_w, fc1_b, conv1_w, conv1_b, conv2_w, conv2_b,
                 conv3_w, conv3_b, proj_w, proj_b, fc2_w, fc2_b):
    assert not np.any(fc1_b), "nonzero fc1_b not supported by this build"
    # merged stage-1 kernel: (c1 + c2 + c3)/3 + identity
    w_eff = np.array(conv3_w[:, 0], dtype=np.float64)            # [CA,3,3,3]
    w_eff[:, :, 1, 1] += conv1_w[:, 0, :, 0, 0]
    w_eff[:, 1, :, :] += conv2_w[:, 0, 0, :, :]
    w_eff /= 3.0
    w_eff[:, 1, 1, 1] += 1.0
    b_eff = np.asarray((conv1_b + conv2_b + conv3_b) / 3.0, np.float64)
    # stage-2 (proj) taps along T + identity
    a_eff = np.array(proj_w[:, 0, :, 0, 0], dtype=np.float64)    # [CA,3]
    a_eff[:, 1] += 1.0

    # block-tridiagonal stationaries: band[k=(c8,t_in), m=(c8,t_out)]
    tt = np.arange(T)
    dtm = tt[:, None] - tt[None, :]                  # t_in - t_out
    mask = np.abs(dtm) <= 1
    dtc = np.clip(dtm + 1, 0, 2)
    # composed per-group stationaries C_w = A_g @ B_{g,w} (stage-2 proj
    # folded into stage 1) and the matching per-partition bias
    band1_arr = np.zeros((NG, 128, 9, 128), np.float64)
    bcg_arr = np.empty((128, NG), np.float64)
    cidx = (np.arange(128) // T)                     # c8 of each partition
    pb = np.asarray(proj_b, np.float64)
    for g in range(NG):
        Ag = np.zeros((128, 128), np.float64)
        for c8 in range(8):
            c = 8 * g + c8
            s = slice(c8 * T, (c8 + 1) * T)
            Ag[s, s] = a_eff[c, dtc] * mask
            for i, (dh, dw) in enumerate(WINS):
                band1_arr[g, s, i, s] = w_eff[c, dtc, dh + 1, dw + 1] * mask
        for i in range(9):
            band1_arr[g, :, i, :] = Ag @ band1_arr[g, :, i, :]
        b1vec = b_eff[8 * g + cidx]
        bcg_arr[:, g] = Ag.T @ b1vec + pb[8 * g + cidx]

    fc1_wT = np.ascontiguousarray(np.asarray(fc1_w, np.float32).T).astype(BF16)
    fc2_wT = np.ascontiguousarray(np.asarray(fc2_w, np.float32).T).astype(BF16)
    return dict(fc1_wT=fc1_wT, fc2_wT=fc2_wT,
                band1=band1_arr.astype(BF16),
                bcg=bcg_arr.astype(np.float32)), np.asarray(fc2_b, np.float32)


def kernel(x, fc1_w, fc1_b, conv1_w, conv1_b, conv2_w, conv2_b,
           conv3_w, conv3_b, proj_w, proj_b, fc2_w, fc2_b, T=16):
    global LAST_EXEC_NS, LAST_RESULTS
    x = np.asarray(x, np.float32)
    Tv = int(np.asarray(T))
    assert Tv == 16 and x.shape == (B * Tv, L, C), (Tv, x.shape)

    if "nc" not in _CACHE:
        _CACHE["nc"] = _build_graph()
    nc = _CACHE["nc"]

    shared, fc2_b_np = _prep_shared(
        np.asarray(fc1_w, np.float32), np.asarray(fc1_b, np.float32),
        np.asarray(conv1_w, np.float32), np.asarray(conv1_b, np.float32),
        np.asarray(conv2_w, np.float32), np.asarray(conv2_b, np.float32),
        np.asarray(conv3_w, np.float32), np.asarray(conv3_b, np.float32),
        np.asarray(proj_w, np.float32), np.asarray(proj_b, np.float32),
        np.asarray(fc2_w, np.float32), np.asarray(fc2_b, np.float32))

    in_maps = []
    for i in range(NCORES):
        clip = x[i * Tv:(i + 1) * Tv]                    # [16, 197, 768]
        tokens = np.ascontiguousarray(clip[:, 1:, :]).reshape(TOK, C)
        m = dict(shared)
        m["xT"] = np.ascontiguousarray(tokens.T).astype(BF16)
        m["xtok"] = (tokens + fc2_b_np[None, :]).astype(BF16)
        in_maps.append(m)

    res = run_bass_kernel_spmd(nc, in_maps, core_ids=list(range(NCORES)),
                               trace=TRACE)
    LAST_EXEC_NS = res.exec_time_ns
    LAST_RESULTS = res

    full = np.array(x)  # CLS rows (and everything) start as x
    for i in range(NCORES):
        h = res.results[i]["out"].astype(np.float32).reshape(Tv, HW, C)
        full[i * Tv:(i + 1) * Tv, 1:, :] = h
    return full


# revision 12
# speedup vs baseline: 1.4842x; 1.1230x over previous
"""Trainium2 Bass kernel for nn_Adapter (ViT video adapter block).

Reference computation (per clip of T=16 frames, 14x14 spatial, 768 ch):
  h   = fc1(x_tokens)                                  # [3136, 384]
  g   = (dw3d_311(h) + dw3d_133(h) + dw3d_333(h))/3 + h
  f   = g + dw3d_311(g)            (proj)
  out = x_tokens + fc2(f)
CLS token passes through unchanged.

Strategy: data-parallel over the 8 clips (B=8), one clip per NeuronCore.
The depthwise 3D convs run entirely on TensorE in a t-on-partition
layout: 48 groups of (16 frames x 8 channels) partitions, spatial plane
on the free dim (zero-halo padded [15 rows x 16 cols]).  The 3 taps
along T collapse into the matmul contraction as block-tridiagonal
128x128 stationaries, so stage 1 is 9 matmuls per group (one per
(dh,dw) window) and stage 2 (proj) is a single banded matmul per group.
Layout transposes between channel-major (fc1/fc2) and t-major (conv)
bounce through DRAM on a single DMA queue (FIFO order enforces the
read-after-write).  fc1/fc2 run on TensorE in bf16; the residual x-add
rides the fc2 PSUM group as an identity matmul.  Output returns bf16.
"""

import os
import sys

sys.path.insert(0, "/opt/trn_rl_repo")

import numpy as np
import ml_dtypes

import concourse.bass as bass
import concourse.mybir as mybir
from concourse import bacc
from concourse.tile import TileContext
from concourse.bass_utils import run_bass_kernel_spmd
from concourse.masks import make_identity


def _install_ntff_hook():
    """Provide antenv.axon_hooks (NTFF profiling hook) if the image lacks
    it, so run_bass_kernel_spmd(trace=True) works.  No-op when present."""
    try:
        import antenv.axon_hooks  # noqa: F401
        return
    except ImportError:
        pass
    import contextlib
    import ctypes
    import types

    so = "/opt/axon/libaxon_pjrt.so"
    try:
        lib = ctypes.CDLL(so)
        lib.axon_start_nrt_profile.argtypes = [
            ctypes.POINTER(ctypes.c_int64), ctypes.c_size_t]
        lib.axon_start_nrt_profile.restype = ctypes.c_int64
        lib.axon_stop_nrt_profile.argtypes = [ctypes.c_char_p]
        lib.axon_stop_nrt_profile.restype = ctypes.c_int64
    except (OSError, AttributeError):
        lib = None

    @contextlib.contextmanager
    def _hook(output_dir, device_ids):
        import jax
        jax.devices()
        ids = ((ctypes.c_int64 * len(device_ids))(*device_ids)
               if device_ids else None)
        rc = lib.axon_start_nrt_profile(ids, len(device_ids or []))
        if rc != 0:
            raise RuntimeError(f"axon_start_nrt_profile rc={rc}")
        try:
            yield
        finally:
            lib.axon_stop_nrt_profile(str(output_dir).encode())

    hook = [_hook if lib is not None else None]
    mod = types.ModuleType("antenv.axon_hooks")
    mod.get_axon_ntff_profile_hook = lambda: hook[0]
    mod.set_axon_ntff_profile_hook = lambda h: hook.__setitem__(0, h)
    sys.modules["antenv.axon_hooks"] = mod


_install_ntff_hook()

BF16 = ml_dtypes.bfloat16

# Problem geometry (hardcoded; kernel must be self-contained).
B, T, L, C, CA = 8, 16, 197, 768, 384
H = W = 14
HW = H * W            # 196
TOK = T * HW          # 3136 tokens per clip
NCORES = 8
NG = 48               # conv groups: 8 channels x 16 frames = 128 partitions
# per-group padded plane: [halo row + 14 data rows] x [14 data + 2 halo]
GH, GW = 15, 16
GPLANE = GH * GW      # 240
PADG = 32             # lead/trail pad inside a group slot (max |off| = 17)
GSLOT = PADG + GPLANE + PADG   # 304
NCH, CHT = 8, 392     # fc1 output chunking

F32 = mybir.dt.float32
BF = mybir.dt.bfloat16

WINS = [(dh, dw) for dh in (-1, 0, 1) for dw in (-1, 0, 1)]

_CACHE = {}

TRACE = os.environ.get("BASS_KERNEL_TRACE", "0") == "1"
LAST_EXEC_NS = None
LAST_RESULTS = None


def _build_graph():
    nc = bacc.Bacc("TRN2", target_bir_lowering=False, debug=False,
                   num_devices=NCORES)

    xT = nc.dram_tensor("xT", [C, TOK], BF, kind="ExternalInput").ap()
    xtok = nc.dram_tensor("xtok", [TOK, C], BF, kind="ExternalInput").ap()
    fc1_wT = nc.dram_tensor("fc1_wT", [C, CA], BF, kind="ExternalInput").ap()
    fc2_wT = nc.dram_tensor("fc2_wT", [CA, C], BF, kind="ExternalInput").ap()
    band1 = nc.dram_tensor("band1", [NG, 128, 9, 128], BF,
                           kind="ExternalInput").ap()
    bcg = nc.dram_tensor("bcg", [128, NG], F32, kind="ExternalInput").ap()
    h_dram = nc.dram_tensor("h_dram", [3, 128, T * 224], BF, kind="Internal").ap()
    f_dram = nc.dram_tensor("f_dram", [3, 128, TOK], BF, kind="Internal").ap()
    out = nc.dram_tensor("out", [TOK, C], BF, kind="ExternalOutput").ap()

    mult = mybir.AluOpType.mult
    add = mybir.AluOpType.add
    IDENT = mybir.ActivationFunctionType.Identity

    with TileContext(nc) as tc:
        with (
            tc.tile_pool(name="persist", bufs=1) as pp,
            tc.tile_pool(name="hh", bufs=2) as hhp,
            tc.tile_pool(name="bg", bufs=6) as bgp,
            tc.tile_pool(name="fg", bufs=3) as fgp,
            tc.tile_pool(name="xm", bufs=6) as xmp,
            tc.tile_pool(name="outs", bufs=3) as outp,
            tc.tile_pool(name="ps1", bufs=2, space="PSUM") as ps1p,
            tc.tile_pool(name="ps2", bufs=3, space="PSUM") as ps2p,
            tc.tile_pool(name="psc", bufs=3, space="PSUM") as pscp,
        ):
            # ---- fc1 weights + chunked xT DMA (n-major so fc1 n=0
            # unblocks after the first 6 small transfers) ----
            fc1w_sb = []
            for k in range(6):
                t = pp.tile([128, CA], BF, tag=f"fc1w{k}", name=f"fc1w{k}")
                nc.gpsimd.dma_start(out=t[:],
                                    in_=fc1_wT[k * 128:(k + 1) * 128, :])
                fc1w_sb.append(t)
            xT_sb = [pp.tile([128, TOK], BF, tag=f"xT{k}", name=f"xT{k}")
                     for k in range(6)]
            for n0, n1 in ((0, 2), (2, 4), (4, 8)):
                for k in range(6):
                    nc.sync.dma_start(
                        out=xT_sb[k][:, n0 * CHT:n1 * CHT],
                        in_=xT[k * 128:(k + 1) * 128, n0 * CHT:n1 * CHT])

            bc_sb = pp.tile([128, NG], F32, tag="bcg", name="bc_sb")
            nc.sync.dma_start(out=bc_sb[:], in_=bcg)

            ident = pp.tile([128, 128], BF, tag="ident", name="ident")
            make_identity(nc, ident[:])
            fc2w_sb = []
            for k in range(3):
                t = pp.tile([128, C], BF, tag=f"fc2w{k}", name=f"fc2w{k}")
                nc.gpsimd.dma_start(out=t[:], in_=fc2_wT[k * 128:(k + 1) * 128, :])
                fc2w_sb.append(t)

            # ---- conv buffers (t-major): halo cells must be zero ----
            G = pp.tile([128, NG * GSLOT], BF, tag="G", name="G")
            nc.vector.memzero(G[:])
            Ft = [pp.tile([128, TOK], BF, tag=f"Ft_{j}", name=f"Ft_{j}")
                  for j in range(3)]

            def data_cells(buf, base):
                """[14,14] data-cell view of a plane at flat `base`."""
                w = buf[:, base + GW:base + GPLANE]
                return w.rearrange("p (h w) -> p h w", h=H, w=GW)[:, :, 0:W]

            def fc1_block(j):
                # padded 16-wide rows: halo cols carry zeros into G
                hj = hhp.tile([128, T * 224], BF, tag="hjt", name=f"H{j}")
                if j < 2:
                    nc.vector.memzero(hj[:])
                for n in range(NCH):
                    ps = ps1p.tile([128, CHT], F32, tag="ps1t",
                                   name=f"ps1_{j}_{n}")
                    for k in range(6):
                        nc.tensor.matmul(
                            ps[:],
                            fc1w_sb[k][:, j * 128:(j + 1) * 128],
                            xT_sb[k][:, n * CHT:(n + 1) * CHT],
                            start=(k == 0), stop=(k == 5),
                        )
                    dst = hj[:, n * 448:(n + 1) * 448].rearrange(
                        "p (t h w) -> p t h w", t=2, h=H, w=GW)[:, :, :, 0:W]
                    nc.scalar.copy(
                        dst, ps[:].rearrange("p (t h w) -> p t h w",
                                             t=2, h=H, w=W))
                return hj

            def g_load4(g0):
                """DRAM h -> t-major G data cells for groups g0..g0+3."""
                j, cb = g0 // 16, (g0 % 16) * 8
                src = h_dram[j, cb:cb + 32, :].rearrange(
                    "(sl c) (t r) -> (c t) sl r", sl=4, c=8, t=T, r=224)
                dst = G[:, g0 * GSLOT:(g0 + 4) * GSLOT].rearrange(
                    "p (sl q) -> p sl q", sl=4, q=GSLOT)[
                    :, :, PADG + GW:PADG + GPLANE]
                nc.gpsimd.dma_start(out=dst, in_=src)

            def conv_group(g, Bg, r):
                gb = g * GSLOT
                pc = pscp.tile([128, GPLANE], F32, tag="psct", name=f"pc1_{g}")
                for i in range(9):
                    dh, dw = WINS[i]
                    off = dh * GW + dw
                    nc.tensor.matmul(
                        pc[:], Bg[:, r, i * 128:(i + 1) * 128],
                        G[:, gb + PADG + off:gb + PADG + off + GPLANE],
                        start=(i == 0), stop=(i == 8),
                    )
                return pc

            def stage2_evict(g, pc2, f8):
                nc.vector.scalar_tensor_tensor(
                    f8[:, (g % 8) * HW:(g % 8 + 1) * HW].rearrange(
                        "p (h w) -> p h w", h=H, w=W),
                    pc2[:, GW:].rearrange("p (h w) -> p h w",
                                          h=H, w=GW)[:, :, 0:W],
                    1.0,
                    bc_sb[:, g:g + 1].broadcast_to([128, H, W]),
                    op0=mult, op1=add)

            def f_flush(b, f8):
                """Write 8 groups of f to DRAM, read back channel-major."""
                j, cb = b // 2, (b % 2) * 64
                dst = f_dram[j, cb:cb + 64, :].rearrange(
                    "(sl c) (t s) -> (c t) sl s", sl=8, c=8, t=T, s=HW)
                nc.scalar.dma_start(
                    out=dst,
                    in_=f8[:].rearrange("p (sl s) -> p sl s", sl=8, s=HW))
                nc.scalar.dma_start(
                    out=Ft[j][cb:cb + 64, :], in_=f_dram[j, cb:cb + 64, :])

            def conv_range(g0, g1):
                f8 = None
                for q in range(g0 // 2, g1 // 2):
                    Bg = bgp.tile([128, 2, 9 * 128], BF, tag="bgt", name=f"Bg_{q}")
                    nc.sync.dma_start(
                        out=Bg[:],
                        in_=band1[2 * q:2 * q + 2].rearrange(
                            "g k w m -> k g (w m)"))
                    for r in range(2):
                        g = 2 * q + r
                        if g % 4 == 0:
                            g_load4(g)
                        if g % 8 == 0:
                            f8 = fgp.tile([128, 8 * HW], BF, tag="f8t", name=f"f8_{g}")
                        pc = conv_group(g, Bg, r)
                        stage2_evict(g, pc, f8)
                        if g % 8 == 7:
                            f_flush(g // 8, f8)

            h0 = fc1_block(0)
            nc.gpsimd.dma_start(out=h_dram[0], in_=h0[:])
            h1 = fc1_block(1)
            conv_range(0, 16)
            nc.gpsimd.dma_start(out=h_dram[1], in_=h1[:])
            h2 = fc1_block(2)
            conv_range(16, 32)
            nc.gpsimd.dma_start(out=h_dram[2], in_=h2[:])
            conv_range(32, 48)

            # ---- fc2 + residual add (identity matmul) + store ----
            m_tiles = [(m * 128, 128) for m in range(24)] + [(3072, 64)]
            for (m0, M) in m_tiles:
                xm = xmp.tile([128, C], BF)
                nc.gpsimd.dma_start(out=xm[:M], in_=xtok[m0:m0 + M, :])
                ot = outp.tile([128, C], BF)
                for nh in range(2):
                    ps = ps2p.tile([128, 384], F32, tag="ps2t",
                                   name=f"ps2_{m0}_{nh}")
                    for k in range(3):
                        nc.tensor.matmul(
                            ps[:M],
                            Ft[k][:, m0:m0 + M],
                            fc2w_sb[k][:, nh * 384:(nh + 1) * 384],
                            start=(k == 0), stop=False,
                        )
                    nc.tensor.matmul(
                        ps[:M], ident[:M, :M],
                        xm[:M, nh * 384:(nh + 1) * 384],
                        start=False, stop=True,
                    )
                    if nh == 0:
                        nc.scalar.copy(ot[:M, 0:384], ps[:M])
                    else:
                        nc.vector.tensor_copy(ot[:M, 384:768], ps[:M])
                nc.scalar.dma_start(out=out[m0:m0 + M, :], in_=ot[:M])

    nc.compile()
    return nc


def _prep_shared(fc1_w, fc1_b, conv1_w, conv1_b, conv2_w, conv2_b,
                 conv3_w, conv3_b, proj_w, proj_b, fc2_w, fc2_b):
    assert not np.any(fc1_b), "nonzero fc1_b not supported by this build"
    # merged stage-1 kernel: (c1 + c2 + c3)/3 + identity
    w_eff = np.array(conv3_w[:, 0], dtype=np.float64)            # [CA,3,3,3]
    w_eff[:, :, 1, 1] += conv1_w[:, 0, :, 0, 0]
    w_eff[:, 1, :, :] += conv2_w[:, 0, 0, :, :]
    w_eff /= 3.0
    w_eff[:, 1, 1, 1] += 1.0
    b_eff = np.asarray((conv1_b + conv2_b + conv3_b) / 3.0, np.float64)
    # stage-2 (proj) taps along T + identity
    a_eff = np.array(proj_w[:, 0, :, 0, 0], dtype=np.float64)    # [CA,3]
    a_eff[:, 1] += 1.0

    # block-tridiagonal stationaries: band[k=(c8,t_in), m=(c8,t_out)]
    tt = np.arange(T)
    dtm = tt[:, None] - tt[None, :]                  # t_in - t_out
    mask = np.abs(dtm) <= 1
    dtc = np.clip(dtm + 1, 0, 2)
    # composed per-group stationaries C_w = A_g @ B_{g,w} (stage-2 proj
    # folded into stage 1) and the matching per-partition bias
    band1_arr = np.zeros((NG, 128, 9, 128), np.float64)
    bcg_arr = np.empty((128, NG), np.float64)
    cidx = (np.arange(128) // T)                     # c8 of each partition
    pb = np.asarray(proj_b, np.float64)
    for g in range(NG):
        Ag = np.zeros((128, 128), np.float64)
        for c8 in range(8):
            c = 8 * g + c8
            s = slice(c8 * T, (c8 + 1) * T)
            Ag[s, s] = a_eff[c, dtc] * mask
            for i, (dh, dw) in enumerate(WINS):
                band1_arr[g, s, i, s] = w_eff[c, dtc, dh + 1, dw + 1] * mask
        for i in range(9):
            band1_arr[g, :, i, :] = Ag @ band1_arr[g, :, i, :]
        b1vec = b_eff[8 * g + cidx]
        bcg_arr[:, g] = Ag.T @ b1vec + pb[8 * g + cidx]

    fc1_wT = np.ascontiguousarray(np.asarray(fc1_w, np.float32).T).astype(BF16)
    fc2_wT = np.ascontiguousarray(np.asarray(fc2_w, np.float32).T).astype(BF16)
    return dict(fc1_wT=fc1_wT, fc2_wT=fc2_wT,
                band1=band1_arr.astype(BF16),
                bcg=bcg_arr.astype(np.float32)), np.asarray(fc2_b, np.float32)


def kernel(x, fc1_w, fc1_b, conv1_w, conv1_b, conv2_w, conv2_b,
           conv3_w, conv3_b, proj_w, proj_b, fc2_w, fc2_b, T=16):
    global LAST_EXEC_NS, LAST_RESULTS
    x = np.asarray(x, np.float32)
    Tv = int(np.asarray(T))
    assert Tv == 16 and x.shape == (B * Tv, L, C), (Tv, x.shape)

    if "nc" not in _CACHE:
        _CACHE["nc"] = _build_graph()
    nc = _CACHE["nc"]

    shared, fc2_b_np = _prep_shared(
        np.asarray(fc1_w, np.float32), np.asarray(fc1_b, np.float32),
        np.asarray(conv1_w, np.float32), np.asarray(conv1_b, np.float32),
        np.asarray(conv2_w, np.float32), np.asarray(conv2_b, np.float32),
        np.asarray(conv3_w, np.float32), np.asarray(conv3_b, np.float32),
        np.asarray(proj_w, np.float32), np.asarray(proj_b, np.float32),
        np.asarray(fc2_w, np.float32), np.asarray(fc2_b, np.float32))

    in_maps = []
    for i in range(NCORES):
        clip = x[i * Tv:(i + 1) * Tv]                    # [16, 197, 768]
        tokens = np.ascontiguousarray(clip[:, 1:, :]).reshape(TOK, C)
        m = dict(shared)
        m["xT"] = np.ascontiguousarray(tokens.T).astype(BF16)
        m["xtok"] = (tokens + fc2_b_np[None, :]).astype(BF16)
        in_maps.append(m)

    res = run_bass_kernel_spmd(nc, in_maps, core_ids=list(range(NCORES)),
                               trace=TRACE)
    LAST_EXEC_NS = res.exec_time_ns
    LAST_RESULTS = res

    full = np.array(x)  # CLS rows (and everything) start as x
    for i in range(NCORES):
        h = res.results[i]["out"].astype(np.float32).reshape(Tv, HW, C)
        full[i * Tv:(i + 1) * Tv, 1:, :] = h
    return full


# revision 13
# speedup vs baseline: 1.5287x; 1.0300x over previous
"""Trainium2 Bass kernel for nn_Adapter (ViT video adapter block).

Reference computation (per clip of T=16 frames, 14x14 spatial, 768 ch):
  h   = fc1(x_tokens)                                  # [3136, 384]
  g   = (dw3d_311(h) + dw3d_133(h) + dw3d_333(h))/3 + h
  f   = g + dw3d_311(g)            (proj)
  out = x_tokens + fc2(f)
CLS token passes through unchanged.

Strategy: data-parallel over the 8 clips (B=8), one clip per NeuronCore.
The depthwise 3D convs run entirely on TensorE in a t-on-partition
layout: 48 groups of (16 frames x 8 channels) partitions, spatial plane
on the free dim (zero-halo padded [15 rows x 16 cols]).  The 3 taps
along T collapse into the matmul contraction as block-tridiagonal
128x128 stationaries, so stage 1 is 9 matmuls per group (one per
(dh,dw) window) and stage 2 (proj) is a single banded matmul per group.
Layout transposes between channel-major (fc1/fc2) and t-major (conv)
bounce through DRAM on a single DMA queue (FIFO order enforces the
read-after-write).  fc1/fc2 run on TensorE in bf16; the residual x-add
rides the fc2 PSUM group as an identity matmul.  Output returns bf16.
"""

import os
import sys

sys.path.insert(0, "/opt/trn_rl_repo")

import numpy as np
import ml_dtypes

import concourse.bass as bass
import concourse.mybir as mybir
from concourse import bacc
from concourse.tile import TileContext
from concourse.bass_utils import run_bass_kernel_spmd
from concourse.masks import make_identity


def _install_ntff_hook():
    """Provide antenv.axon_hooks (NTFF profiling hook) if the image lacks
    it, so run_bass_kernel_spmd(trace=True) works.  No-op when present."""
    try:
        import antenv.axon_hooks  # noqa: F401
        return
    except ImportError:
        pass
    import contextlib
    import ctypes
    import types

    so = "/opt/axon/libaxon_pjrt.so"
    try:
        lib = ctypes.CDLL(so)
        lib.axon_start_nrt_profile.argtypes = [
            ctypes.POINTER(ctypes.c_int64), ctypes.c_size_t]
        lib.axon_start_nrt_profile.restype = ctypes.c_int64
        lib.axon_stop_nrt_profile.argtypes = [ctypes.c_char_p]
        lib.axon_stop_nrt_profile.restype = ctypes.c_int64
    except (OSError, AttributeError):
        lib = None

    @contextlib.contextmanager
    def _hook(output_dir, device_ids):
        import jax
        jax.devices()
        ids = ((ctypes.c_int64 * len(device_ids))(*device_ids)
               if device_ids else None)
        rc = lib.axon_start_nrt_profile(ids, len(device_ids or []))
        if rc != 0:
            raise RuntimeError(f"axon_start_nrt_profile rc={rc}")
        try:
            yield
        finally:
            lib.axon_stop_nrt_profile(str(output_dir).encode())

    hook = [_hook if lib is not None else None]
    mod = types.ModuleType("antenv.axon_hooks")
    mod.get_axon_ntff_profile_hook = lambda: hook[0]
    mod.set_axon_ntff_profile_hook = lambda h: hook.__setitem__(0, h)
    sys.modules["antenv.axon_hooks"] = mod


_install_ntff_hook()

BF16 = ml_dtypes.bfloat16

# Problem geometry (hardcoded; kernel must be self-contained).
B, T, L, C, CA = 8, 16, 197, 768, 384
H = W = 14
HW = H * W            # 196
TOK = T * HW          # 3136 tokens per clip
NCORES = 8
NG = 48               # conv groups: 8 channels x 16 frames = 128 partitions
# per-group padded plane: [halo row + 14 data rows] x [14 data + 2 halo]
GH, GW = 15, 16
GPLANE = GH * GW      # 240
PADG = 32             # lead/trail pad inside a group slot (max |off| = 17)
GSLOT = PADG + GPLANE + PADG   # 304
NCH, CHT = 8, 392     # fc1 output chunking

F32 = mybir.dt.float32
BF = mybir.dt.bfloat16

WINS = [(dh, dw) for dh in (-1, 0, 1) for dw in (-1, 0, 1)]

_CACHE = {}

TRACE = os.environ.get("BASS_KERNEL_TRACE", "0") == "1"
LAST_EXEC_NS = None
LAST_RESULTS = None


def _build_graph():
    nc = bacc.Bacc("TRN2", target_bir_lowering=False, debug=False,
                   num_devices=NCORES)

    xT = nc.dram_tensor("xT", [C, TOK], BF, kind="ExternalInput").ap()
    xtok = nc.dram_tensor("xtok", [TOK, C], BF, kind="ExternalInput").ap()
    fc1_wT = nc.dram_tensor("fc1_wT", [C, CA], BF, kind="ExternalInput").ap()
    fc2_wT = nc.dram_tensor("fc2_wT", [CA, C], BF, kind="ExternalInput").ap()
    band1 = nc.dram_tensor("band1", [NG, 128, 9, 128], mybir.dt.float8e4,
                           kind="ExternalInput").ap()
    bcg = nc.dram_tensor("bcg", [128, NG], F32, kind="ExternalInput").ap()
    h_dram = nc.dram_tensor("h_dram", [3, 128, T * 224], BF, kind="Internal").ap()
    f_dram = nc.dram_tensor("f_dram", [3, 128, TOK], BF, kind="Internal").ap()
    out = nc.dram_tensor("out", [TOK, C], BF, kind="ExternalOutput").ap()

    mult = mybir.AluOpType.mult
    add = mybir.AluOpType.add
    IDENT = mybir.ActivationFunctionType.Identity

    with TileContext(nc) as tc:
        with (
            tc.tile_pool(name="persist", bufs=1) as pp,
            tc.tile_pool(name="hh", bufs=3) as hhp,
            tc.tile_pool(name="bg", bufs=10) as bgp,
            tc.tile_pool(name="fg", bufs=3) as fgp,
            tc.tile_pool(name="xm", bufs=6) as xmp,
            tc.tile_pool(name="outs", bufs=3) as outp,
            tc.tile_pool(name="ps1", bufs=2, space="PSUM") as ps1p,
            tc.tile_pool(name="ps2", bufs=3, space="PSUM") as ps2p,
            tc.tile_pool(name="psc", bufs=3, space="PSUM") as pscp,
        ):
            # ---- fc1 weights + chunked xT DMA (n-major so fc1 n=0
            # unblocks after the first 6 small transfers) ----
            fc1w_sb = []
            for k in range(6):
                t = pp.tile([128, CA], BF, tag=f"fc1w{k}", name=f"fc1w{k}")
                nc.gpsimd.dma_start(out=t[:],
                                    in_=fc1_wT[k * 128:(k + 1) * 128, :])
                fc1w_sb.append(t)
            xT_sb = [pp.tile([128, TOK], BF, tag=f"xT{k}", name=f"xT{k}")
                     for k in range(6)]
            for n0, n1 in ((0, 2), (2, 4), (4, 8)):
                for k in range(6):
                    nc.sync.dma_start(
                        out=xT_sb[k][:, n0 * CHT:n1 * CHT],
                        in_=xT[k * 128:(k + 1) * 128, n0 * CHT:n1 * CHT])

            bc_sb = pp.tile([128, NG], F32, tag="bcg", name="bc_sb")
            nc.sync.dma_start(out=bc_sb[:], in_=bcg)

            ident = pp.tile([128, 128], BF, tag="ident", name="ident")
            make_identity(nc, ident[:])
            fc2w_sb = []
            for k in range(3):
                t = pp.tile([128, C], BF, tag=f"fc2w{k}", name=f"fc2w{k}")
                nc.gpsimd.dma_start(out=t[:], in_=fc2_wT[k * 128:(k + 1) * 128, :])
                fc2w_sb.append(t)

            # ---- conv buffers (t-major): halo cells must be zero ----
            G = pp.tile([128, NG * GSLOT], BF, tag="G", name="G")
            nc.vector.memzero(G[:])
            Ft = [pp.tile([128, TOK], BF, tag=f"Ft_{j}", name=f"Ft_{j}")
                  for j in range(3)]

            def data_cells(buf, base):
                """[14,14] data-cell view of a plane at flat `base`."""
                w = buf[:, base + GW:base + GPLANE]
                return w.rearrange("p (h w) -> p h w", h=H, w=GW)[:, :, 0:W]

            def fc1_block(j):
                # padded 16-wide rows: halo cols carry zeros into G
                hj = hhp.tile([128, T * 224], BF, tag="hjt", name=f"H{j}")
                if j < 2:
                    nc.vector.memzero(hj[:])
                for n in range(NCH):
                    ps = ps1p.tile([128, CHT], F32, tag="ps1t",
                                   name=f"ps1_{j}_{n}")
                    for k in range(6):
                        nc.tensor.matmul(
                            ps[:],
                            fc1w_sb[k][:, j * 128:(j + 1) * 128],
                            xT_sb[k][:, n * CHT:(n + 1) * CHT],
                            start=(k == 0), stop=(k == 5),
                        )
                    dst = hj[:, n * 448:(n + 1) * 448].rearrange(
                        "p (t h w) -> p t h w", t=2, h=H, w=GW)[:, :, :, 0:W]
                    nc.scalar.copy(
                        dst, ps[:].rearrange("p (t h w) -> p t h w",
                                             t=2, h=H, w=W))
                return hj

            def g_load4(g0):
                """DRAM h -> t-major G data cells for groups g0..g0+3."""
                j, cb = g0 // 16, (g0 % 16) * 8
                src = h_dram[j, cb:cb + 32, :].rearrange(
                    "(sl c) (t r) -> (c t) sl r", sl=4, c=8, t=T, r=224)
                dst = G[:, g0 * GSLOT:(g0 + 4) * GSLOT].rearrange(
                    "p (sl q) -> p sl q", sl=4, q=GSLOT)[
                    :, :, PADG + GW:PADG + GPLANE]
                nc.gpsimd.dma_start(out=dst, in_=src)

            def conv_group(g, Bg, r):
                gb = g * GSLOT
                pc = pscp.tile([128, GPLANE], F32, tag="psct", name=f"pc1_{g}")
                for i in range(9):
                    dh, dw = WINS[i]
                    off = dh * GW + dw
                    nc.tensor.matmul(
                        pc[:], Bg[:, r, i * 128:(i + 1) * 128],
                        G[:, gb + PADG + off:gb + PADG + off + GPLANE],
                        start=(i == 0), stop=(i == 8),
                    )
                return pc

            def stage2_evict(g, pc2, f8):
                nc.vector.scalar_tensor_tensor(
                    f8[:, (g % 8) * HW:(g % 8 + 1) * HW].rearrange(
                        "p (h w) -> p h w", h=H, w=W),
                    pc2[:, GW:].rearrange("p (h w) -> p h w",
                                          h=H, w=GW)[:, :, 0:W],
                    1.0,
                    bc_sb[:, g:g + 1].broadcast_to([128, H, W]),
                    op0=mult, op1=add)

            def f_flush(b, f8):
                """Write 8 groups of f to DRAM, read back channel-major."""
                j, cb = b // 2, (b % 2) * 64
                dst = f_dram[j, cb:cb + 64, :].rearrange(
                    "(sl c) (t s) -> (c t) sl s", sl=8, c=8, t=T, s=HW)
                nc.scalar.dma_start(
                    out=dst,
                    in_=f8[:].rearrange("p (sl s) -> p sl s", sl=8, s=HW))
                nc.scalar.dma_start(
                    out=Ft[j][cb:cb + 64, :], in_=f_dram[j, cb:cb + 64, :])

            def conv_range(g0, g1):
                f8 = None
                for q in range(g0 // 2, g1 // 2):
                    Bg = bgp.tile([128, 2, 9 * 128], mybir.dt.float8e4,
                                  tag="bgt", name=f"Bg_{q}")
                    nc.sync.dma_start(
                        out=Bg[:],
                        in_=band1[2 * q:2 * q + 2].rearrange(
                            "g k w m -> k g (w m)"))
                    for r in range(2):
                        g = 2 * q + r
                        if g % 4 == 0:
                            g_load4(g)
                        if g % 8 == 0:
                            f8 = fgp.tile([128, 8 * HW], BF, tag="f8t", name=f"f8_{g}")
                        pc = conv_group(g, Bg, r)
                        stage2_evict(g, pc, f8)
                        if g % 8 == 7:
                            f_flush(g // 8, f8)

            h0 = fc1_block(0)
            nc.gpsimd.dma_start(out=h_dram[0], in_=h0[:])
            h1 = fc1_block(1)
            conv_range(0, 16)
            nc.gpsimd.dma_start(out=h_dram[1], in_=h1[:])
            h2 = fc1_block(2)
            conv_range(16, 32)
            nc.gpsimd.dma_start(out=h_dram[2], in_=h2[:])
            conv_range(32, 48)

            # ---- fc2 + residual add (identity matmul) + store ----
            m_tiles = [(m * 128, 128) for m in range(24)] + [(3072, 64)]
            for (m0, M) in m_tiles:
                xm = xmp.tile([128, C], BF)
                nc.gpsimd.dma_start(out=xm[:M], in_=xtok[m0:m0 + M, :])
                ot = outp.tile([128, C], BF)
                for nh in range(2):
                    ps = ps2p.tile([128, 384], F32, tag="ps2t",
                                   name=f"ps2_{m0}_{nh}")
                    for k in range(3):
                        nc.tensor.matmul(
                            ps[:M],
                            Ft[k][:, m0:m0 + M],
                            fc2w_sb[k][:, nh * 384:(nh + 1) * 384],
                            start=(k == 0), stop=False,
                        )
                    nc.tensor.matmul(
                        ps[:M], ident[:M, :M],
                        xm[:M, nh * 384:(nh + 1) * 384],
                        start=False, stop=True,
                    )
                    if nh == 0:
                        nc.scalar.copy(ot[:M, 0:384], ps[:M])
                    else:
                        nc.vector.tensor_copy(ot[:M, 384:768], ps[:M])
                nc.scalar.dma_start(out=out[m0:m0 + M, :], in_=ot[:M])

    nc.compile()
    return nc


def _prep_shared(fc1_w, fc1_b, conv1_w, conv1_b, conv2_w, conv2_b,
                 conv3_w, conv3_b, proj_w, proj_b, fc2_w, fc2_b):
    assert not np.any(fc1_b), "nonzero fc1_b not supported by this build"
    # merged stage-1 kernel: (c1 + c2 + c3)/3 + identity
    w_eff = np.array(conv3_w[:, 0], dtype=np.float64)            # [CA,3,3,3]
    w_eff[:, :, 1, 1] += conv1_w[:, 0, :, 0, 0]
    w_eff[:, 1, :, :] += conv2_w[:, 0, 0, :, :]
    w_eff /= 3.0
    w_eff[:, 1, 1, 1] += 1.0
    b_eff = np.asarray((conv1_b + conv2_b + conv3_b) / 3.0, np.float64)
    # stage-2 (proj) taps along T + identity
    a_eff = np.array(proj_w[:, 0, :, 0, 0], dtype=np.float64)    # [CA,3]
    a_eff[:, 1] += 1.0

    # block-tridiagonal stationaries: band[k=(c8,t_in), m=(c8,t_out)]
    tt = np.arange(T)
    dtm = tt[:, None] - tt[None, :]                  # t_in - t_out
    mask = np.abs(dtm) <= 1
    dtc = np.clip(dtm + 1, 0, 2)
    # composed per-group stationaries C_w = A_g @ B_{g,w} (stage-2 proj
    # folded into stage 1) and the matching per-partition bias
    band1_arr = np.zeros((NG, 128, 9, 128), np.float64)
    bcg_arr = np.empty((128, NG), np.float64)
    cidx = (np.arange(128) // T)                     # c8 of each partition
    pb = np.asarray(proj_b, np.float64)
    for g in range(NG):
        Ag = np.zeros((128, 128), np.float64)
        for c8 in range(8):
            c = 8 * g + c8
            s = slice(c8 * T, (c8 + 1) * T)
            Ag[s, s] = a_eff[c, dtc] * mask
            for i, (dh, dw) in enumerate(WINS):
                band1_arr[g, s, i, s] = w_eff[c, dtc, dh + 1, dw + 1] * mask
        for i in range(9):
            band1_arr[g, :, i, :] = Ag @ band1_arr[g, :, i, :]
        b1vec = b_eff[8 * g + cidx]
        bcg_arr[:, g] = Ag.T @ b1vec + pb[8 * g + cidx]

    fc1_wT = np.ascontiguousarray(np.asarray(fc1_w, np.float32).T).astype(BF16)
    fc2_wT = np.ascontiguousarray(np.asarray(fc2_w, np.float32).T).astype(BF16)
    return dict(fc1_wT=fc1_wT, fc2_wT=fc2_wT,
                band1=band1_arr.astype(ml_dtypes.float8_e4m3fn),
                bcg=bcg_arr.astype(np.float32)), np.asarray(fc2_b, np.float32)


def kernel(x, fc1_w, fc1_b, conv1_w, conv1_b, conv2_w, conv2_b,
           conv3_w, conv3_b, proj_w, proj_b, fc2_w, fc2_b, T=16):
    global LAST_EXEC_NS, LAST_RESULTS
    x = np.asarray(x, np.float32)
    Tv = int(np.asarray(T))
    assert Tv == 16 and x.shape == (B * Tv, L, C), (Tv, x.shape)

    if "nc" not in _CACHE:
        _CACHE["nc"] = _build_graph()
    nc = _CACHE["nc"]

    shared, fc2_b_np = _prep_shared(
        np.asarray(fc1_w, np.float32), np.asarray(fc1_b, np.float32),
        np.asarray(conv1_w, np.float32), np.asarray(conv1_b, np.float32),
        np.asarray(conv2_w, np.float32), np.asarray(conv2_b, np.float32),
        np.asarray(conv3_w, np.float32), np.asarray(conv3_b, np.float32),
        np.asarray(proj_w, np.float32), np.asarray(proj_b, np.float32),
        np.asarray(fc2_w, np.float32), np.asarray(fc2_b, np.float32))

    in_maps = []
    for i in range(NCORES):
        clip = x[i * Tv:(i + 1) * Tv]                    # [16, 197, 768]
        tokens = np.ascontiguousarray(clip[:, 1:, :]).reshape(TOK, C)
        m = dict(shared)
        m["xT"] = np.ascontiguousarray(tokens.T).astype(BF16)
        m["xtok"] = (tokens + fc2_b_np[None, :]).astype(BF16)
        in_maps.append(m)

    res = run_bass_kernel_spmd(nc, in_maps, core_ids=list(range(NCORES)),
                               trace=TRACE)
    LAST_EXEC_NS = res.exec_time_ns
    LAST_RESULTS = res

    full = np.array(x)  # CLS rows (and everything) start as x
    for i in range(NCORES):
        h = res.results[i]["out"].astype(np.float32).reshape(Tv, HW, C)
        full[i * Tv:(i + 1) * Tv, 1:, :] = h
    return full
